# revision 34
# baseline (speedup 1.0000x reference)
"""GQA attention kernel for Trainium2, sharded over 8 NeuronCores.

Problem (hardcoded): B=4, S=1024, HID=2048, 16 query heads, 4 KV heads,
head_dim=128, RoPE (base 10000), causal softmax, O-projection.

Sharding: core c handles (batch b = c//2, head-half = c%2): 8 query heads,
2 KV heads, and the matching column/row shards of Wq/Wk/Wv/Wo. Each core
produces a partial O-projection output [S, HID]; the host sums the two
halves per batch element.

v3 (from trace analysis of v2 @286us):
- scores->exp restructured into 5 ragged PSUM groups per head, each exp'd
  with ONE wide scalar ACTIVATE ([128,1024] across 2 banks) instead of 12
  narrow ones: scalar per head drops 7.4us -> 5.3us and scores matmuls no
  longer recycle PSUM banks at scalar speed.
- software pipeline deepened to +2 heads: Q-proj of head h+2 is interleaved
  between score groups of head h, so the PE never waits on exp/dn/recip.
  Per-head PE work ~11.4us runs back-to-back -> HAM stays at 8/8.
- RoPE partition-rotate done by two SBUF->SBUF DMAs (swap 64-partition
  halves) instead of a P64 perm matmul: frees 1024 PE cyc/head and lets the
  sin-mul read SBUF bf16 at 2x DVE rate instead of PSUM f32 at 1x.
- PSUM budget: ps_sc [128,1024]x2 (scores groups), ps_q [128,512]x4
  (qproj sc0/sc1, dn dps, ctx j0/j1 ring) = exactly 8 banks, ring orders
  chosen so no allocation ever waits on a later-program-order op.
- prologue DMA over 4 queues (sync/gpsimd/scalar/vector) ordered so the
  first pass-1 matmul can start as soon as ~1MB lands.
- O-proj PSUM pool bufs=8 + stores round-robin sync/gpsimd.

Per-core layouts (partition dim first, 128 everywhere):
  xT   [128, kt(16), s(1024)]   xT[p,kt,s] = x[s, 128kt+p]          bf16
  wq   [128, h(8), kt(16), d(128)]                                  bf16
  wk/wv[128, kt(16), c(2)*d(128)]                                   bf16
  wo   [128, h(8), e(2048)]     wo[p,h,e] = Wo[1024*half+128h+p, e] bf16
  qT/kT[128d, s]   roped in T orientation (DMA half-swap + 3 DVE ops)
  v    [128s, st(8)*c(2)*d(128)]  natural, for ctx stationary
  eS   [128k, ragged q spans]   exp(scores^T) bf16, 4608 cols
  ctxT [128d, s] per head       normalized context, feeds O-proj
"""
import math
from contextlib import ExitStack

import numpy as np
import ml_dtypes

import concourse.bass as bass
import concourse.bacc as bacc
import concourse.tile as tile
from concourse import mybir
from concourse.bass_utils import run_bass_kernel_spmd

F32 = mybir.dt.float32
BF16 = mybir.dt.bfloat16
F8E4 = mybir.dt.float8e4
QSCALE = 64.0  # host premultiplies Wq by this before fp8; folded into exp

B, S, HID = 4, 1024, 2048
NH, NKV, D = 16, 4, 128
HPC = 8          # query heads per core
KVPC = 2         # kv heads per core
SCALE = 1.0 / math.sqrt(D)
NKT = HID // 128  # 16 contraction tiles
NST = S // 128    # 8 sequence tiles
KVD = KVPC * D    # 256

# eS ragged packing: block for key-tile kt covers q in [128kt, S), stored at
# eS col ES_OFF[kt] + (q - 128kt).
ES_OFF = []
_o = 0
for _kt in range(NST):
    ES_OFF.append(_o)
    _o += S - 128 * _kt
ES_W = _o  # 4608

# exp groups: eS col ranges, each <=1024 wide (one [128,1024] 2-bank PSUM
# tile + one wide ACTIVATE). Pieces never cross a 512 (bank) boundary.
GB = [0, 1024, 2048, 3072, 4096, ES_W]
NG = 5


def _group_pieces(g):
    glo, ghi = GB[g], GB[g + 1]
    out = []
    for kt in range(NST):
        blo = ES_OFF[kt]
        bhi = blo + (S - 128 * kt)
        lo, hi = max(blo, glo), min(bhi, ghi)
        if lo >= hi:
            continue
        p = lo
        while p < hi:
            nxt = min(hi, (p // 512 + 1) * 512)
            out.append((kt, 128 * kt + (p - blo), 128 * kt + (nxt - blo),
                        p - glo))
            p = nxt
    return out


GROUP_PIECES = [_group_pieces(g) for g in range(NG)]
# O-proj tiles whose heads 0-5 partial is precomputed during heads 6/7
PRE_TILES = [(st, ec) for st in range(5) for ec in range(2)]
# which kts have their (diagonal-masked) block start inside group g
DIAG_G = [[kt for kt in range(NST) if GB[g] <= ES_OFF[kt] < GB[g + 1]]
          for g in range(NG)]

# denominator accumulation pieces: per kt, the q-span [128kt, S) split at the
# j boundary (q=512, the dn PSUM bank edge). Each piece is issued after the
# exp group that contains its last eS column (groups complete in order).
# kt0 pieces carry start=True (kt0 spans all q, so it initializes both
# banks); the final writer of each bank carries stop=True.
DN_G = [[] for _ in range(NG)]
for _kt in range(NST):
    _spans = [(128 * _kt, 512), (512, S)] if 128 * _kt < 512 else \
        [(128 * _kt, S)]
    for (_q0, _q1) in _spans:
        _lastcol = ES_OFF[_kt] + (_q1 - 1) - 128 * _kt
        _g = next(g for g in range(NG) if _lastcol < GB[g + 1])
        _stop = (_kt == 3 and _q1 == 512) or (_kt == 7)
        DN_G[_g].append((_kt, _q0, _q1, _kt == 0, _stop))


def build_kernel():
    nc = bacc.Bacc(None)
    xT = nc.dram_tensor("xT", [128, NKT * S], BF16, kind="ExternalInput")
    # fp8 copies for the Q projection (DoubleRow: 2 k-tiles per pass)
    xT8 = nc.dram_tensor("xT8", [128, NKT // 2, 2, S], F8E4,
                         kind="ExternalInput")
    wq8 = nc.dram_tensor("wq8", [128, HPC * NKT, D], F8E4,
                         kind="ExternalInput")
    wk = nc.dram_tensor("wk", [128, NKT * KVD], BF16, kind="ExternalInput")
    wv = nc.dram_tensor("wv", [128, NKT * KVD], BF16, kind="ExternalInput")
    wo = nc.dram_tensor("wo", [128, HPC * HID], BF16, kind="ExternalInput")
    # tables bundled: cosT(1024) | sinTs(1024) | p64(128, unused) | dmask(128)
    tbl = nc.dram_tensor("tbl", [128, 2304], BF16, kind="ExternalInput")
    out = nc.dram_tensor("out", [S, HID], F32, kind="ExternalOutput")
    # partial O-proj (heads 0-5) for 10 tiles, computed during heads 6/7 in
    # the PE slots freed by the ended Q-proj pipeline; host adds this into
    # out[0:640, 0:1024]
    out2 = nc.dram_tensor("out2", [5 * 128, 2 * 512], F32,
                          kind="ExternalOutput")

    with tile.TileContext(nc) as tc, ExitStack() as top:
        const = top.enter_context(tc.tile_pool(name="const", bufs=1))
        xk_pool = top.enter_context(tc.tile_pool(name="xk", bufs=1))
        wkv_pool = top.enter_context(tc.tile_pool(name="wkv", bufs=1))
        kT_pool = top.enter_context(tc.tile_pool(name="kT", bufs=1))
        v_pool = top.enter_context(tc.tile_pool(name="v", bufs=1))
        ctxT_pool = top.enter_context(tc.tile_pool(name="ctxT", bufs=1))
        wq_pool = top.enter_context(tc.tile_pool(name="wq", bufs=4))
        qT_pool = top.enter_context(tc.tile_pool(name="qT", bufs=3))
        eS_pool = top.enter_context(tc.tile_pool(name="eS", bufs=2))
        rc_pool = top.enter_context(tc.tile_pool(name="rc", bufs=2))
        rot_pool = top.enter_context(tc.tile_pool(name="rot", bufs=2))
        tmp_pool = top.enter_context(tc.tile_pool(name="tmp", bufs=2))
        wo_pool = top.enter_context(tc.tile_pool(name="wo", bufs=1))
        out_pool = top.enter_context(tc.tile_pool(name="outp", bufs=4))

        # ---- input DMA over the 3 DMA-capable queues (sync/gpsimd/scalar);
        # first pass-1 matmul needs only wk_h0 (scalar q), wv_h0 (gpsimd q),
        # xkp0 (sync q) — each is the first transfer on its queue ----------
        wk_sb = wkv_pool.tile([128, NKT * KVD], BF16, tag="wk")
        wv_sb = wkv_pool.tile([128, NKT * KVD], BF16, tag="wv")
        HK = NKT * KVD // 2
        nc.scalar.dma_start(wk_sb[:, 0:HK], wk[:, 0:HK])
        nc.gpsimd.dma_start(wv_sb[:, 0:HK], wv[:, 0:HK])

        xkp = [xk_pool.tile([128, 2 * S], BF16, tag=f"xkp{g}", name=f"xkp{g}")
               for g in range(NKT // 2)]
        # fp8 x copy for Q-proj, first needed at qproj(0) right after pass 1;
        # interleave its chunks into the bf16 stream so they land in time
        xkp8 = [xk_pool.tile([128, 2, S], F8E4, tag=f"xkp8_{g}",
                             name=f"xkp8_{g}") for g in range(NKT // 2)]
        for g in range(NKT // 2):
            eng = nc.sync if g % 2 == 0 else nc.gpsimd
            eng.dma_start(xkp[g][:], xT[:, 2 * g * S:(2 * g + 2) * S])
            if g >= 2:
                eng.dma_start(xkp8[g - 2][:], xT8[:, g - 2, :, :])
        for g in range(NKT // 2 - 2, NKT // 2):
            eng = nc.sync if g % 2 == 0 else nc.gpsimd
            eng.dma_start(xkp8[g][:], xT8[:, g, :, :])

        # keep sync/gpsimd dedicated to x during pass 1; everything else
        # (needed from pass 2 onward) streams on the scalar queue in
        # first-use order
        wq_sb = {}
        for h in range(4):
            wq_sb[h] = wq_pool.tile([128, NKT, D], F8E4, tag="wqh",
                                    name=f"wqh{h}")
        nc.scalar.dma_start(wk_sb[:, HK:], wk[:, HK:])
        nc.scalar.dma_start(wv_sb[:, HK:], wv[:, HK:])
        nc.scalar.dma_start(wq_sb[0][:], wq8[:, 0:NKT, :])
        tbl_sb = const.tile([128, 2304], BF16)
        nc.scalar.dma_start(tbl_sb[:], tbl[:])
        nc.scalar.dma_start(wq_sb[1][:], wq8[:, NKT:2 * NKT, :])
        nc.scalar.dma_start(wq_sb[2][:], wq8[:, 2 * NKT:3 * NKT, :])
        nc.scalar.dma_start(wq_sb[3][:], wq8[:, 3 * NKT:4 * NKT, :])

        def xkc(kt, a, b):
            g, r = kt // 2, kt % 2
            return xkp[g][:, r * S + a:r * S + b]

        cosT_sb = tbl_sb[:, 0:S]
        sinTs_sb = tbl_sb[:, S:2 * S]
        dmask_sb = tbl_sb[:, 2 * S + D:2 * S + 2 * D]

        ones_bf = const.tile([128, 1], BF16)
        nc.vector.memset(ones_bf[:], 1.0)

        ctxT = [ctxT_pool.tile([D, S], BF16, tag=f"ctxT{h}", name=f"ctxT{h}")
                for h in range(HPC)]
        kT = [kT_pool.tile([D, S], BF16, tag=f"kT{c}", name=f"kT{c}")
              for c in range(KVPC)]
        v_sb = v_pool.tile([128, NST * KVD], BF16)

        # ---- pass 1: stream kt chunks once -> all of K proj + V st0-3 ----
        with ExitStack() as pro:
            psP = pro.enter_context(tc.tile_pool(name="psP", bufs=1,
                                                 space="PSUM"))
            kps = {}
            for c in range(KVPC):
                for sc in range(2):
                    kps[(c, sc)] = psP.tile([128, 512], F32, tag=f"kp{c}{sc}",
                                            name=f"kp{c}{sc}")
            vps = [psP.tile([128, 512], F32, tag=f"vp{st}", name=f"vp{st}")
                   for st in range(4)]
            for kt in range(NKT):
                for c in range(KVPC):
                    for sc in range(2):
                        nc.tensor.matmul(
                            kps[(c, sc)][:],
                            wk_sb[:, kt * KVD + c * D:kt * KVD + (c + 1) * D],
                            xkc(kt, sc * 512, (sc + 1) * 512),
                            start=(kt == 0), stop=(kt == NKT - 1))
                for st in range(4):
                    nc.tensor.matmul(
                        vps[st][:, 0:KVD], xkc(kt, st * 128, (st + 1) * 128),
                        wv_sb[:, kt * KVD:(kt + 1) * KVD],
                        start=(kt == 0), stop=(kt == NKT - 1))
            for c in range(KVPC):
                for sc in range(2):
                    nc.vector.tensor_copy(kT[c][:, sc * 512:(sc + 1) * 512],
                                          kps[(c, sc)][:])
            for st in range(4):
                nc.vector.tensor_copy(v_sb[:, st * KVD:(st + 1) * KVD],
                                      vps[st][:, 0:KVD])

        # ---- pass 2 + head loop PSUM pools: exactly 8 banks --------------
        with ExitStack() as mid:
            ps_sc = mid.enter_context(tc.tile_pool(name="ps_sc", bufs=2,
                                                   space="PSUM"))
            ps_q = mid.enter_context(tc.tile_pool(name="ps_q", bufs=2,
                                                  space="PSUM"))
            ps_dn = mid.enter_context(tc.tile_pool(name="ps_dn", bufs=1,
                                                   space="PSUM"))

            def rope_inplace(dst, rope_id, tag="rot", eng=None, ve=None):
                """RoPE in T orientation on a [128, S] bf16 tile in place.
                Partition rotate-by-64 via two SBUF->SBUF DMAs (sign folded
                into sinTs), then 3 elementwise ops at bf16 SBUF rate."""
                eng = eng or nc.sync
                ve = ve or nc.vector
                rot = rot_pool.tile([128, S], BF16, tag=tag,
                                    name=f"rot{rope_id}")
                eng.dma_start(rot[0:64, :], dst[64:128, :])
                eng.dma_start(rot[64:128, :], dst[0:64, :])
                tmp = tmp_pool.tile([128, S], BF16, tag=f"rt_{tag}",
                                    name=f"rtmp{rope_id}")
                ve.tensor_mul(tmp[:], rot[:], sinTs_sb)
                ve.tensor_mul(dst[:], dst[:], cosT_sb)
                ve.tensor_add(dst[:], dst[:], tmp[:])

            # pass 2: K rope -> fp8 Q proj heads 0/1 -> V st4-7, so the
            # qT copy + rope latency hides under the V matmuls. K ropes get
            # dedicated rot tiles + the scalar DMA queue (free here) so no
            # ring WAR chains them to the qT ropes.
            for c in range(KVPC):
                rope_inplace(kT[c], f"k{c}", tag=f"rotk{c}", eng=nc.gpsimd,
                             ve=nc.gpsimd)

            qT = {}

            def qproj_psum(hq, sc, p0, p1, ps):
                # fp8 DoubleRow: each matmul contracts a PAIR of k-tiles
                # (2*128 rows) at 2 MACs/cell/cycle
                for p in range(p0, p1):
                    nc.tensor.matmul(
                        ps[:], wq_sb[hq][:, 2 * p:2 * p + 2, :],
                        xkp8[p][:, :, sc * 512:(sc + 1) * 512],
                        start=(p == 0), stop=(p == NKT // 2 - 1),
                        perf_mode=mybir.MatmulPerfMode.DoubleRow)

            for hq in range(2):
                qT[hq] = qT_pool.tile([D, S], BF16, tag="qTh",
                                      name=f"qT{hq}")
                for sc in range(2):
                    ps = ps_q.tile([128, 512], F32, tag="ps_q",
                                   name=f"q{hq}s{sc}")
                    qproj_psum(hq, sc, 0, NKT // 2, ps)
                    nc.scalar.copy(qT[hq][:, sc * 512:(sc + 1) * 512], ps[:])
                rope_inplace(qT[hq], f"q{hq}", eng=nc.scalar)


            # ---- head loop with 2-group scores lookahead: G0/G1 of head
            # h+1 are issued near the end of head h so its exp (the scalar
            # critical path) gets a head start --------------------------------
            eS_t = {}

            def scores_group(h, g):
                eS = eS_t[h]
                c = h // (HPC // KVPC)
                w = GB[g + 1] - GB[g]
                ps = ps_sc.tile([128, 1024], F32, tag="ps_sc",
                                name=f"sc{h}_{g}")
                for (kt, q0, q1, rel) in GROUP_PIECES[g]:
                    nc.tensor.matmul(
                        ps[:, rel:rel + q1 - q0],
                        kT[c][:, kt * 128:(kt + 1) * 128],
                        qT[h][:, q0:q1], start=True, stop=True)
                nc.scalar.activation(
                    eS[:, GB[g]:GB[g + 1]], ps[:, 0:w],
                    mybir.ActivationFunctionType.Exp, scale=SCALE / QSCALE)
                for kt in DIAG_G[g]:
                    off = ES_OFF[kt]
                    nc.vector.tensor_mul(eS[:, off:off + 128],
                                         eS[:, off:off + 128], dmask_sb)

            def begin_head(h):
                eS_t[h] = eS_pool.tile([128, ES_W], BF16, tag="eS",
                                       name=f"eS{h}")
                scores_group(h, 0)
                scores_group(h, 1)

            begin_head(0)
            for st in range(4, NST):
                ps = ps_q.tile([128, 512], F32, tag="ps_q", name=f"vq{st}")
                for kt in range(NKT):
                    nc.tensor.matmul(
                        ps[:, 0:KVD], xkc(kt, st * 128, (st + 1) * 128),
                        wv_sb[:, kt * KVD:(kt + 1) * KVD],
                        start=(kt == 0), stop=(kt == NKT - 1))
                nc.vector.tensor_copy(v_sb[:, st * KVD:(st + 1) * KVD],
                                      ps[:, 0:KVD])
            wo_sb = None
            for h in range(HPC):
                c = h // (HPC // KVPC)  # local kv head
                hq = h + 2              # head whose Q-proj we compute now
                eS = eS_t[h]

                if hq < HPC:
                    qT[hq] = qT_pool.tile([D, S], BF16, tag="qTh",
                                          name=f"qT{hq}")
                    psq0 = ps_q.tile([128, 512], F32, tag="ps_q",
                                     name=f"q{hq}s0")
                dn = ps_dn.tile([128, 1024], F32, tag="dn", name=f"dn{h}")

                def op_partial(k):
                    # heads 6/7: partial O-proj (heads 0-5) in the PE slots
                    # the Q-proj pipeline no longer needs
                    st, ec = PRE_TILES[(h - 6) * 5 + k]
                    po2 = ps_q.tile([128, 512], F32, tag="ps_q",
                                    name=f"op{st}_{ec}")
                    for hh in range(6):
                        nc.tensor.matmul(
                            po2[:], ctxT[hh][:, st * 128:(st + 1) * 128],
                            wo_sb[:, hh * HID + ec * 512:
                                  hh * HID + (ec + 1) * 512],
                            start=(hh == 0), stop=(hh == 5))
                    ot2 = out_pool.tile([128, 512], F32, tag="ot",
                                        name=f"ot2_{st}_{ec}")
                    dst = out2[st * 128:(st + 1) * 128,
                               ec * 512:(ec + 1) * 512]
                    if k % 2:
                        nc.scalar.copy(ot2[:], po2[:])
                        nc.gpsimd.dma_start(dst, ot2[:])
                    else:
                        nc.vector.tensor_copy(ot2[:], po2[:])
                        nc.sync.dma_start(dst, ot2[:])

                def dn_group(g):
                    # denominator partial sums on the PE: ones-matmul per
                    # piece, PSUM-accumulated into dn[0, q0:q1]
                    for (kt, q0, q1, st_, sp_) in DN_G[g]:
                        nc.tensor.matmul(
                            dn[:1, q0:q1], ones_bf[:],
                            eS[:, ES_OFF[kt] + q0 - 128 * kt:
                               ES_OFF[kt] + q1 - 128 * kt],
                            start=st_, stop=sp_)

                scores_group(h, 2)
                if hq < HPC:
                    qproj_psum(hq, 0, 0, 4, psq0)
                else:
                    op_partial(0)
                dn_group(0)
                scores_group(h, 3)
                if hq < HPC:
                    qproj_psum(hq, 0, 4, 6, psq0)
                else:
                    op_partial(1)
                dn_group(1)
                scores_group(h, 4)
                if hq < HPC:
                    qproj_psum(hq, 0, 6, 8, psq0)
                    # scalar queue: lands after the exp activates of g2-g4
                    nc.scalar.copy(qT[hq][:, 0:512], psq0[:])
                    psq1 = ps_q.tile([128, 512], F32, tag="ps_q",
                                     name=f"q{hq}s1")
                    qproj_psum(hq, 1, 0, 4, psq1)
                else:
                    op_partial(2)
                dn_group(2)
                dn_group(3)
                dn_group(4)
                if hq < HPC:
                    qproj_psum(hq, 1, 4, 8, psq1)
                    nc.scalar.copy(qT[hq][:, 512:1024], psq1[:])
                else:
                    op_partial(3)

                if h + 1 < HPC:
                    begin_head(h + 1)

                rc = rc_pool.tile([1, S], F32, tag="rc", name=f"rc{h}")
                nc.vector.reciprocal_approx_fast(rc[:1, 0:512],
                                                 dn[:1, 0:512])
                rb0 = tmp_pool.tile([128, 512], F32, tag="rbtmp",
                                    name=f"rb0_{h}")
                nc.gpsimd.partition_broadcast(rb0[:], rc[:1, 0:512])
                nc.vector.reciprocal_approx_fast(rc[:1, 512:1024],
                                                 dn[:1, 512:1024])
                rb1 = tmp_pool.tile([128, 512], F32, tag="rbtmp",
                                    name=f"rb1_{h}")
                nc.gpsimd.partition_broadcast(rb1[:], rc[:1, 512:1024])

                if hq >= HPC:
                    op_partial(4)

                # ctx matmuls (ragged accumulate) into one [128,1024] ps_sc
                # tile (keeps the ps_q ring free of the normalize chain)
                pcx = ps_sc.tile([128, 1024], F32, tag="ps_sc",
                                 name=f"pc_{h}")
                for j in range(2):
                    kts = [kt for kt in range(NST)
                           if max(128 * kt, j * 512) < (j + 1) * 512]
                    for kt in kts:
                        qlo = 128 * kt
                        lo = max(qlo, j * 512)
                        hi = (j + 1) * 512
                        nc.tensor.matmul(
                            pcx[:, lo:hi],
                            v_sb[:, kt * KVD + c * D:kt * KVD + (c + 1) * D],
                            eS[:, ES_OFF[kt] + lo - qlo:ES_OFF[kt] + hi - qlo],
                            start=(kt == kts[0]), stop=(kt == kts[-1]))
                nc.vector.tensor_mul(ctxT[h][:, 0:512], pcx[:, 0:512],
                                     rb0[:])
                nc.vector.tensor_mul(ctxT[h][:, 512:1024], pcx[:, 512:1024],
                                     rb1[:])

                if hq < HPC:
                    rope_inplace(qT[hq], f"q{hq}")

                # staged weight prefetches
                if h < 4 and h + 4 < HPC:
                    wq_sb[h + 4] = wq_pool.tile([128, NKT, D], F8E4,
                                                tag="wqh", name=f"wqh{h+4}")
                    nc.gpsimd.dma_start(
                        wq_sb[h + 4][:],
                        wq8[:, (h + 4) * NKT:(h + 5) * NKT, :])
                if h == 2:  # wo arrives while attention still running
                    wo_sb = wo_pool.tile([128, HPC * HID], BF16)
                    HW2 = HPC * HID // 2
                    nc.sync.dma_start(wo_sb[:, 0:HW2], wo[:, 0:HW2])
                    nc.gpsimd.dma_start(wo_sb[:, HW2:], wo[:, HW2:])

        # ---- O projection: own 8-bank PSUM pool, deep pipeline -----------
        with ExitStack() as fin:
            psO = fin.enter_context(tc.tile_pool(name="psO", bufs=8,
                                                 space="PSUM"))
            pre = set(PRE_TILES)
            for st in range(NST):
                for ec in range(HID // 512):
                    po = psO.tile([128, 512], F32, tag="po",
                                  name=f"po{st}_{ec}")
                    h0 = 6 if (st, ec) in pre else 0
                    for h in range(h0, HPC):
                        nc.tensor.matmul(
                            po[:], ctxT[h][:, st * 128:(st + 1) * 128],
                            wo_sb[:, h * HID + ec * 512:h * HID + (ec + 1) * 512],
                            start=(h == h0), stop=(h == HPC - 1))
                    ot = out_pool.tile([128, 512], F32, tag="ot")
                    if (st * 4 + ec) % 2:
                        nc.scalar.copy(ot[:], po[:])
                        nc.gpsimd.dma_start(
                            out[st * 128:(st + 1) * 128,
                                ec * 512:(ec + 1) * 512], ot[:])
                    else:
                        nc.vector.tensor_copy(ot[:], po[:])
                        nc.sync.dma_start(
                            out[st * 128:(st + 1) * 128,
                                ec * 512:(ec + 1) * 512], ot[:])
    nc.finalize()
    return nc


def host_prep(hidden_states, Wq, Wk, Wv, Wo):
    """Pre-transpose/cast/relayout all inputs on the host (bf16 + fp8)."""
    bf = ml_dtypes.bfloat16
    f8 = ml_dtypes.float8_e4m3fn
    xTs, xT8s = [], []
    for b in range(B):
        t = hidden_states[b].T.reshape(NKT, 128, S).transpose(1, 0, 2)
        xTs.append(np.ascontiguousarray(t.astype(bf)).reshape(128, NKT * S))
        xT8s.append(np.ascontiguousarray(
            np.clip(t, -240, 240).astype(f8)).reshape(128, NKT * S))
    halves = []
    for hf in range(2):
        wqh = Wq[:, 1024 * hf:1024 * (hf + 1)].reshape(NKT, 128, HPC, D)
        wqh = np.ascontiguousarray(
            np.clip(wqh.transpose(1, 2, 0, 3) * QSCALE, -240, 240)
            .astype(f8)).reshape(128, HPC * NKT * D)
        wkh = Wk[:, KVD * hf:KVD * (hf + 1)].reshape(NKT, 128, KVD)
        wkh = np.ascontiguousarray(
            wkh.transpose(1, 0, 2).astype(bf)).reshape(128, NKT * KVD)
        wvh = Wv[:, KVD * hf:KVD * (hf + 1)].reshape(NKT, 128, KVD)
        wvh = np.ascontiguousarray(
            wvh.transpose(1, 0, 2).astype(bf)).reshape(128, NKT * KVD)
        woh = Wo[1024 * hf:1024 * (hf + 1), :].reshape(HPC, 128, HID)
        woh = np.ascontiguousarray(
            woh.transpose(1, 0, 2).astype(bf)).reshape(128, HPC * HID)
        halves.append((wqh, wkh, wvh, woh))

    inv_freq = 1.0 / (10000.0 ** (np.arange(0, D, 2, dtype=np.float64) / D))
    t = np.arange(S, dtype=np.float64)
    freqs = np.outer(t, inv_freq)
    emb = np.concatenate([freqs, freqs], -1)
    cosT = np.cos(emb).T
    sinTs_f = np.sin(emb).T.copy()
    sinTs_f[:64] *= -1.0
    p64 = np.zeros((D, D), dtype=np.float64)
    for d in range(D):
        p64[d, (d + 64) % D] = 1.0
    dmask = np.triu(np.ones((128, 128), dtype=np.float64))
    tbl = np.ascontiguousarray(
        np.concatenate([cosT, sinTs_f, p64, dmask], axis=1)).astype(bf)
    return xTs, xT8s, halves, tbl


_CACHE = {}


def kernel(hidden_states, Wq, Wk, Wv, Wo, _trace=False, _tmpdir=None):
    hidden_states = np.ascontiguousarray(hidden_states, dtype=np.float32)
    Wq = np.ascontiguousarray(Wq, dtype=np.float32)
    Wk = np.ascontiguousarray(Wk, dtype=np.float32)
    Wv = np.ascontiguousarray(Wv, dtype=np.float32)
    Wo = np.ascontiguousarray(Wo, dtype=np.float32)

    if "nc" not in _CACHE:
        _CACHE["nc"] = build_kernel()
    nc = _CACHE["nc"]
    xTs, xT8s, halves, tbl = host_prep(hidden_states, Wq, Wk, Wv, Wo)

    in_maps = []
    for cid in range(8):
        b, hf = cid // 2, cid % 2
        wqh, wkh, wvh, woh = halves[hf]
        in_maps.append({
            "xT": xTs[b], "xT8": xT8s[b], "wq8": wqh, "wk": wkh, "wv": wvh,
            "wo": woh, "tbl": tbl,
        })
    res = run_bass_kernel_spmd(nc, in_maps, list(range(8)),
                               trace=_trace, tmpdir=_tmpdir)
    out = np.zeros((B, S, HID), dtype=np.float32)
    for cid in range(8):
        out[cid // 2] += res.results[cid]["out"]
        out[cid // 2][0:640, 0:1024] += res.results[cid]["out2"]
    if _trace:
        return out, res
    return out


# revision 35
# speedup vs baseline: 1.0866x; 1.0866x over previous
"""GQA attention kernel for Trainium2, sharded over 8 NeuronCores.

Problem (hardcoded): B=4, S=1024, HID=2048, 16 query heads, 4 KV heads,
head_dim=128, RoPE (base 10000), causal softmax, O-projection.

Sharding: core c handles (batch b = c//2, head-half = c%2): 8 query heads,
2 KV heads, and the matching column/row shards of Wq/Wk/Wv/Wo. Each core
produces a partial O-projection output [S, HID]; the host sums the two
halves per batch element.

v3 (from trace analysis of v2 @286us):
- scores->exp restructured into 5 ragged PSUM groups per head, each exp'd
  with ONE wide scalar ACTIVATE ([128,1024] across 2 banks) instead of 12
  narrow ones: scalar per head drops 7.4us -> 5.3us and scores matmuls no
  longer recycle PSUM banks at scalar speed.
- software pipeline deepened to +2 heads: Q-proj of head h+2 is interleaved
  between score groups of head h, so the PE never waits on exp/dn/recip.
  Per-head PE work ~11.4us runs back-to-back -> HAM stays at 8/8.
- RoPE partition-rotate done by two SBUF->SBUF DMAs (swap 64-partition
  halves) instead of a P64 perm matmul: frees 1024 PE cyc/head and lets the
  sin-mul read SBUF bf16 at 2x DVE rate instead of PSUM f32 at 1x.
- PSUM budget: ps_sc [128,1024]x2 (scores groups), ps_q [128,512]x4
  (qproj sc0/sc1, dn dps, ctx j0/j1 ring) = exactly 8 banks, ring orders
  chosen so no allocation ever waits on a later-program-order op.
- prologue DMA over 4 queues (sync/gpsimd/scalar/vector) ordered so the
  first pass-1 matmul can start as soon as ~1MB lands.
- O-proj PSUM pool bufs=8 + stores round-robin sync/gpsimd.

Per-core layouts (partition dim first, 128 everywhere):
  xT   [128, kt(16), s(1024)]   xT[p,kt,s] = x[s, 128kt+p]          bf16
  wq   [128, h(8), kt(16), d(128)]                                  bf16
  wk/wv[128, kt(16), c(2)*d(128)]                                   bf16
  wo   [128, h(8), e(2048)]     wo[p,h,e] = Wo[1024*half+128h+p, e] bf16
  qT/kT[128d, s]   roped in T orientation (DMA half-swap + 3 DVE ops)
  v    [128s, st(8)*c(2)*d(128)]  natural, for ctx stationary
  eS   [128k, ragged q spans]   exp(scores^T) bf16, 4608 cols
  ctxT [128d, s] per head       normalized context, feeds O-proj
"""
import math
from contextlib import ExitStack

import numpy as np
import ml_dtypes

import concourse.bass as bass
import concourse.bacc as bacc
import concourse.tile as tile
from concourse import mybir
from concourse.bass_utils import run_bass_kernel_spmd

F32 = mybir.dt.float32
BF16 = mybir.dt.bfloat16
F8E4 = mybir.dt.float8e4
QSCALE = 64.0  # host premultiplies Wq by this before fp8; folded into exp

B, S, HID = 4, 1024, 2048
NH, NKV, D = 16, 4, 128
HPC = 8          # query heads per core
KVPC = 2         # kv heads per core
SCALE = 1.0 / math.sqrt(D)
NKT = HID // 128  # 16 contraction tiles
NST = S // 128    # 8 sequence tiles
KVD = KVPC * D    # 256

# eS ragged packing: block for key-tile kt covers q in [128kt, S), stored at
# eS col ES_OFF[kt] + (q - 128kt).
ES_OFF = []
_o = 0
for _kt in range(NST):
    ES_OFF.append(_o)
    _o += S - 128 * _kt
ES_W = _o  # 4608

# exp groups: eS col ranges, each <=1024 wide (one [128,1024] 2-bank PSUM
# tile + one wide ACTIVATE). Pieces never cross a 512 (bank) boundary.
GB = [0, 1024, 2048, 3072, 4096, ES_W]
NG = 5


def _group_pieces(g):
    glo, ghi = GB[g], GB[g + 1]
    out = []
    for kt in range(NST):
        blo = ES_OFF[kt]
        bhi = blo + (S - 128 * kt)
        lo, hi = max(blo, glo), min(bhi, ghi)
        if lo >= hi:
            continue
        p = lo
        while p < hi:
            nxt = min(hi, (p // 512 + 1) * 512)
            out.append((kt, 128 * kt + (p - blo), 128 * kt + (nxt - blo),
                        p - glo))
            p = nxt
    return out


GROUP_PIECES = [_group_pieces(g) for g in range(NG)]
# O-proj tiles whose heads 0-5 partial is precomputed during heads 6/7
PRE_TILES = [(st, ec) for st in range(5) for ec in range(2)]
# which kts have their (diagonal-masked) block start inside group g
DIAG_G = [[kt for kt in range(NST) if GB[g] <= ES_OFF[kt] < GB[g + 1]]
          for g in range(NG)]

# denominator accumulation pieces: per kt, the q-span [128kt, S) split at the
# j boundary (q=512, the dn PSUM bank edge). Each piece is issued after the
# exp group that contains its last eS column (groups complete in order).
# kt0 pieces carry start=True (kt0 spans all q, so it initializes both
# banks); the final writer of each bank carries stop=True.
DN_G = [[] for _ in range(NG)]
for _kt in range(NST):
    _spans = [(128 * _kt, 512), (512, S)] if 128 * _kt < 512 else \
        [(128 * _kt, S)]
    for (_q0, _q1) in _spans:
        _lastcol = ES_OFF[_kt] + (_q1 - 1) - 128 * _kt
        _g = next(g for g in range(NG) if _lastcol < GB[g + 1])
        _stop = (_kt == 3 and _q1 == 512) or (_kt == 7)
        DN_G[_g].append((_kt, _q0, _q1, _kt == 0, _stop))


def build_kernel():
    nc = bacc.Bacc(None)
    xT = nc.dram_tensor("xT", [128, NKT * S], BF16, kind="ExternalInput")
    # fp8 copies for the Q projection (DoubleRow: 2 k-tiles per pass)
    xT8 = nc.dram_tensor("xT8", [128, NKT // 2, 2, S], F8E4,
                         kind="ExternalInput")
    wq8 = nc.dram_tensor("wq8", [128, HPC * NKT, D], F8E4,
                         kind="ExternalInput")
    wk = nc.dram_tensor("wk", [128, NKT * KVD], BF16, kind="ExternalInput")
    wv = nc.dram_tensor("wv", [128, NKT * KVD], BF16, kind="ExternalInput")
    wo = nc.dram_tensor("wo", [128, HPC * HID], BF16, kind="ExternalInput")
    # tables bundled: cosT(1024) | sinTs(1024) | p64(128, unused) | dmask(128)
    tbl = nc.dram_tensor("tbl", [128, 2304], BF16, kind="ExternalInput")
    out = nc.dram_tensor("out", [S, HID], F32, kind="ExternalOutput")
    # partial O-proj (heads 0-5) for 10 tiles, computed during heads 6/7 in
    # the PE slots freed by the ended Q-proj pipeline; host adds this into
    # out[0:640, 0:1024]
    out2 = nc.dram_tensor("out2", [5 * 128, 2 * 512], F32,
                          kind="ExternalOutput")

    with tile.TileContext(nc) as tc, ExitStack() as top:
        const = top.enter_context(tc.tile_pool(name="const", bufs=1))
        xk_pool = top.enter_context(tc.tile_pool(name="xk", bufs=1))
        wkv_pool = top.enter_context(tc.tile_pool(name="wkv", bufs=1))
        kT_pool = top.enter_context(tc.tile_pool(name="kT", bufs=1))
        v_pool = top.enter_context(tc.tile_pool(name="v", bufs=1))
        ctxT_pool = top.enter_context(tc.tile_pool(name="ctxT", bufs=1))
        wq_pool = top.enter_context(tc.tile_pool(name="wq", bufs=4))
        qT_pool = top.enter_context(tc.tile_pool(name="qT", bufs=3))
        eS_pool = top.enter_context(tc.tile_pool(name="eS", bufs=2))
        rc_pool = top.enter_context(tc.tile_pool(name="rc", bufs=2))
        tmp_pool = top.enter_context(tc.tile_pool(name="tmp", bufs=2))
        wo_pool = top.enter_context(tc.tile_pool(name="wo", bufs=1))
        out_pool = top.enter_context(tc.tile_pool(name="outp", bufs=4))

        # ---- input DMA over the 3 DMA-capable queues (sync/gpsimd/scalar);
        # first pass-1 matmul needs only wk_h0 (scalar q), wv_h0 (gpsimd q),
        # xkp0 (sync q) — each is the first transfer on its queue ----------
        wk_sb = wkv_pool.tile([128, NKT * KVD], BF16, tag="wk")
        wv_sb = wkv_pool.tile([128, NKT * KVD], BF16, tag="wv")
        HK = NKT * KVD // 2
        nc.scalar.dma_start(wk_sb[:, 0:HK], wk[:, 0:HK])
        nc.gpsimd.dma_start(wv_sb[:, 0:HK], wv[:, 0:HK])

        xkp = [xk_pool.tile([128, 2 * S], BF16, tag=f"xkp{g}", name=f"xkp{g}")
               for g in range(NKT // 2)]
        # fp8 x copy for Q-proj, first needed at qproj(0) right after pass 1;
        # interleave its chunks into the bf16 stream so they land in time
        xkp8 = [xk_pool.tile([128, 2, S], F8E4, tag=f"xkp8_{g}",
                             name=f"xkp8_{g}") for g in range(NKT // 2)]
        for g in range(NKT // 2):
            eng = nc.sync if g % 2 == 0 else nc.gpsimd
            eng.dma_start(xkp[g][:], xT[:, 2 * g * S:(2 * g + 2) * S])
            if g >= 2:
                eng.dma_start(xkp8[g - 2][:], xT8[:, g - 2, :, :])
        for g in range(NKT // 2 - 2, NKT // 2):
            eng = nc.sync if g % 2 == 0 else nc.gpsimd
            eng.dma_start(xkp8[g][:], xT8[:, g, :, :])

        # keep sync/gpsimd dedicated to x during pass 1; everything else
        # (needed from pass 2 onward) streams on the scalar queue in
        # first-use order
        wq_sb = {}
        for h in range(4):
            wq_sb[h] = wq_pool.tile([128, NKT, D], F8E4, tag="wqh",
                                    name=f"wqh{h}")
        nc.scalar.dma_start(wk_sb[:, HK:], wk[:, HK:])
        nc.scalar.dma_start(wv_sb[:, HK:], wv[:, HK:])
        nc.scalar.dma_start(wq_sb[0][:], wq8[:, 0:NKT, :])
        tbl_sb = const.tile([128, 2304], BF16)
        nc.scalar.dma_start(tbl_sb[:], tbl[:])
        nc.scalar.dma_start(wq_sb[1][:], wq8[:, NKT:2 * NKT, :])
        nc.scalar.dma_start(wq_sb[2][:], wq8[:, 2 * NKT:3 * NKT, :])
        nc.scalar.dma_start(wq_sb[3][:], wq8[:, 3 * NKT:4 * NKT, :])

        def xkc(kt, a, b):
            g, r = kt // 2, kt % 2
            return xkp[g][:, r * S + a:r * S + b]

        cosT_sb = tbl_sb[:, 0:S]
        sinTs_sb = tbl_sb[:, S:2 * S]
        p64_sb = tbl_sb[:, 2 * S:2 * S + D]
        dmask_sb = tbl_sb[:, 2 * S + D:2 * S + 2 * D]

        ones_bf = const.tile([128, 1], BF16)
        nc.vector.memset(ones_bf[:], 1.0)

        ctxT = [ctxT_pool.tile([D, S], BF16, tag=f"ctxT{h}", name=f"ctxT{h}")
                for h in range(HPC)]
        kT = [kT_pool.tile([D, S], BF16, tag=f"kT{c}", name=f"kT{c}")
              for c in range(KVPC)]
        v_sb = v_pool.tile([128, NST * KVD], BF16)

        # ---- pass 1: stream kt chunks once -> all of K proj + V st0-3 ----
        with ExitStack() as pro:
            psP = pro.enter_context(tc.tile_pool(name="psP", bufs=1,
                                                 space="PSUM"))
            kps = {}
            for c in range(KVPC):
                for sc in range(2):
                    kps[(c, sc)] = psP.tile([128, 512], F32, tag=f"kp{c}{sc}",
                                            name=f"kp{c}{sc}")
            vps = [psP.tile([128, 512], F32, tag=f"vp{st}", name=f"vp{st}")
                   for st in range(4)]
            for kt in range(NKT):
                for c in range(KVPC):
                    for sc in range(2):
                        nc.tensor.matmul(
                            kps[(c, sc)][:],
                            wk_sb[:, kt * KVD + c * D:kt * KVD + (c + 1) * D],
                            xkc(kt, sc * 512, (sc + 1) * 512),
                            start=(kt == 0), stop=(kt == NKT - 1))
                for st in range(4):
                    nc.tensor.matmul(
                        vps[st][:, 0:KVD], xkc(kt, st * 128, (st + 1) * 128),
                        wv_sb[:, kt * KVD:(kt + 1) * KVD],
                        start=(kt == 0), stop=(kt == NKT - 1))
            for c in range(KVPC):
                for sc in range(2):
                    nc.vector.tensor_copy(kT[c][:, sc * 512:(sc + 1) * 512],
                                          kps[(c, sc)][:])
            for st in range(4):
                nc.vector.tensor_copy(v_sb[:, st * KVD:(st + 1) * KVD],
                                      vps[st][:, 0:KVD])

        # ---- pass 2 + head loop PSUM pools: exactly 8 banks --------------
        with ExitStack() as mid:
            ps_sc = mid.enter_context(tc.tile_pool(name="ps_sc", bufs=2,
                                                   space="PSUM"))
            ps_q = mid.enter_context(tc.tile_pool(name="ps_q", bufs=2,
                                                  space="PSUM"))
            ps_dn = mid.enter_context(tc.tile_pool(name="ps_dn", bufs=1,
                                                   space="PSUM"))

            def rope_inplace(dst, rope_id):
                """RoPE in T orientation on a [128, S] bf16 tile in place.
                Partition rotate-by-64 via a P64 perm matmul into the ps_dn
                bank pair (idle between denominator uses; sign of the
                rotate is folded into sinTs), then 3 DVE ops."""
                rot = ps_dn.tile([128, 1024], F32, tag="dn",
                                 name=f"rot{rope_id}")
                for sc in range(2):
                    nc.tensor.matmul(rot[:, sc * 512:(sc + 1) * 512], p64_sb,
                                     dst[:, sc * 512:(sc + 1) * 512],
                                     start=True, stop=True)
                tmp = tmp_pool.tile([128, S], BF16, tag="ropetmp",
                                    name=f"rtmp{rope_id}")
                nc.vector.tensor_mul(tmp[:], rot[:], sinTs_sb)
                nc.vector.tensor_mul(dst[:], dst[:], cosT_sb)
                nc.vector.tensor_add(dst[:], dst[:], tmp[:])

            # pass 2: K rope -> fp8 Q proj heads 0/1 -> V st4-7, so the
            # qT copy + rope latency hides under the V matmuls. K ropes get
            # dedicated rot tiles + the scalar DMA queue (free here) so no
            # ring WAR chains them to the qT ropes.
            for c in range(KVPC):
                rope_inplace(kT[c], f"k{c}")

            qT = {}

            def qproj_psum(hq, sc, p0, p1, ps):
                # fp8 DoubleRow: each matmul contracts a PAIR of k-tiles
                # (2*128 rows) at 2 MACs/cell/cycle
                for p in range(p0, p1):
                    nc.tensor.matmul(
                        ps[:], wq_sb[hq][:, 2 * p:2 * p + 2, :],
                        xkp8[p][:, :, sc * 512:(sc + 1) * 512],
                        start=(p == 0), stop=(p == NKT // 2 - 1),
                        perf_mode=mybir.MatmulPerfMode.DoubleRow)

            for hq in range(2):
                qT[hq] = qT_pool.tile([D, S], BF16, tag="qTh",
                                      name=f"qT{hq}")
                for sc in range(2):
                    ps = ps_q.tile([128, 512], F32, tag="ps_q",
                                   name=f"q{hq}s{sc}")
                    qproj_psum(hq, sc, 0, NKT // 2, ps)
                    nc.scalar.copy(qT[hq][:, sc * 512:(sc + 1) * 512], ps[:])
                rope_inplace(qT[hq], f"q{hq}")


            # ---- head loop with 2-group scores lookahead: G0/G1 of head
            # h+1 are issued near the end of head h so its exp (the scalar
            # critical path) gets a head start --------------------------------
            eS_t = {}

            def scores_group(h, g):
                eS = eS_t[h]
                c = h // (HPC // KVPC)
                w = GB[g + 1] - GB[g]
                ps = ps_sc.tile([128, 1024], F32, tag="ps_sc",
                                name=f"sc{h}_{g}")
                for (kt, q0, q1, rel) in GROUP_PIECES[g]:
                    nc.tensor.matmul(
                        ps[:, rel:rel + q1 - q0],
                        kT[c][:, kt * 128:(kt + 1) * 128],
                        qT[h][:, q0:q1], start=True, stop=True)
                nc.scalar.activation(
                    eS[:, GB[g]:GB[g + 1]], ps[:, 0:w],
                    mybir.ActivationFunctionType.Exp, scale=SCALE / QSCALE)
                for kt in DIAG_G[g]:
                    off = ES_OFF[kt]
                    nc.vector.tensor_mul(eS[:, off:off + 128],
                                         eS[:, off:off + 128], dmask_sb)

            def begin_head(h):
                eS_t[h] = eS_pool.tile([128, ES_W], BF16, tag="eS",
                                       name=f"eS{h}")
                scores_group(h, 0)
                scores_group(h, 1)

            begin_head(0)
            for st in range(4, NST):
                ps = ps_q.tile([128, 512], F32, tag="ps_q", name=f"vq{st}")
                for kt in range(NKT):
                    nc.tensor.matmul(
                        ps[:, 0:KVD], xkc(kt, st * 128, (st + 1) * 128),
                        wv_sb[:, kt * KVD:(kt + 1) * KVD],
                        start=(kt == 0), stop=(kt == NKT - 1))
                nc.vector.tensor_copy(v_sb[:, st * KVD:(st + 1) * KVD],
                                      ps[:, 0:KVD])
            wo_sb = None
            for h in range(HPC):
                c = h // (HPC // KVPC)  # local kv head
                hq = h + 2              # head whose Q-proj we compute now
                eS = eS_t[h]

                if hq < HPC:
                    qT[hq] = qT_pool.tile([D, S], BF16, tag="qTh",
                                          name=f"qT{hq}")
                    psq0 = ps_q.tile([128, 512], F32, tag="ps_q",
                                     name=f"q{hq}s0")
                dn = ps_dn.tile([128, 1024], F32, tag="dn", name=f"dn{h}")

                def op_partial(k):
                    # heads 6/7: partial O-proj (heads 0-5) in the PE slots
                    # the Q-proj pipeline no longer needs
                    st, ec = PRE_TILES[(h - 6) * 5 + k]
                    po2 = ps_q.tile([128, 512], F32, tag="ps_q",
                                    name=f"op{st}_{ec}")
                    for hh in range(6):
                        nc.tensor.matmul(
                            po2[:], ctxT[hh][:, st * 128:(st + 1) * 128],
                            wo_sb[:, hh * HID + ec * 512:
                                  hh * HID + (ec + 1) * 512],
                            start=(hh == 0), stop=(hh == 5))
                    ot2 = out_pool.tile([128, 512], F32, tag="ot",
                                        name=f"ot2_{st}_{ec}")
                    dst = out2[st * 128:(st + 1) * 128,
                               ec * 512:(ec + 1) * 512]
                    if k % 2:
                        nc.scalar.copy(ot2[:], po2[:])
                        nc.gpsimd.dma_start(dst, ot2[:])
                    else:
                        nc.vector.tensor_copy(ot2[:], po2[:])
                        nc.sync.dma_start(dst, ot2[:])

                def dn_group(g):
                    # denominator partial sums on the PE: ones-matmul per
                    # piece, PSUM-accumulated into dn[0, q0:q1]
                    for (kt, q0, q1, st_, sp_) in DN_G[g]:
                        nc.tensor.matmul(
                            dn[:1, q0:q1], ones_bf[:],
                            eS[:, ES_OFF[kt] + q0 - 128 * kt:
                               ES_OFF[kt] + q1 - 128 * kt],
                            start=st_, stop=sp_)

                scores_group(h, 2)
                if hq < HPC:
                    qproj_psum(hq, 0, 0, 4, psq0)
                else:
                    op_partial(0)
                dn_group(0)
                scores_group(h, 3)
                if hq < HPC:
                    qproj_psum(hq, 0, 4, 6, psq0)
                else:
                    op_partial(1)
                dn_group(1)
                scores_group(h, 4)
                if hq < HPC:
                    qproj_psum(hq, 0, 6, 8, psq0)
                    # scalar queue: lands after the exp activates of g2-g4
                    nc.scalar.copy(qT[hq][:, 0:512], psq0[:])
                    psq1 = ps_q.tile([128, 512], F32, tag="ps_q",
                                     name=f"q{hq}s1")
                    qproj_psum(hq, 1, 0, 4, psq1)
                else:
                    op_partial(2)
                dn_group(2)
                dn_group(3)
                dn_group(4)
                if hq < HPC:
                    qproj_psum(hq, 1, 4, 8, psq1)
                    nc.scalar.copy(qT[hq][:, 512:1024], psq1[:])
                else:
                    op_partial(3)

                if h + 1 < HPC:
                    begin_head(h + 1)

                rc = rc_pool.tile([1, S], F32, tag="rc", name=f"rc{h}")
                nc.vector.reciprocal_approx_fast(rc[:1, 0:512],
                                                 dn[:1, 0:512])
                rb0 = tmp_pool.tile([128, 512], F32, tag="rbtmp",
                                    name=f"rb0_{h}")
                nc.gpsimd.partition_broadcast(rb0[:], rc[:1, 0:512])
                nc.vector.reciprocal_approx_fast(rc[:1, 512:1024],
                                                 dn[:1, 512:1024])
                rb1 = tmp_pool.tile([128, 512], F32, tag="rbtmp",
                                    name=f"rb1_{h}")
                nc.gpsimd.partition_broadcast(rb1[:], rc[:1, 512:1024])

                if hq >= HPC:
                    op_partial(4)

                # ctx matmuls (ragged accumulate) into one [128,1024] ps_sc
                # tile (keeps the ps_q ring free of the normalize chain)
                pcx = ps_sc.tile([128, 1024], F32, tag="ps_sc",
                                 name=f"pc_{h}")
                for j in range(2):
                    kts = [kt for kt in range(NST)
                           if max(128 * kt, j * 512) < (j + 1) * 512]
                    for kt in kts:
                        qlo = 128 * kt
                        lo = max(qlo, j * 512)
                        hi = (j + 1) * 512
                        nc.tensor.matmul(
                            pcx[:, lo:hi],
                            v_sb[:, kt * KVD + c * D:kt * KVD + (c + 1) * D],
                            eS[:, ES_OFF[kt] + lo - qlo:ES_OFF[kt] + hi - qlo],
                            start=(kt == kts[0]), stop=(kt == kts[-1]))
                nc.vector.tensor_mul(ctxT[h][:, 0:512], pcx[:, 0:512],
                                     rb0[:])
                nc.vector.tensor_mul(ctxT[h][:, 512:1024], pcx[:, 512:1024],
                                     rb1[:])

                if hq < HPC:
                    rope_inplace(qT[hq], f"q{hq}")

                # staged weight prefetches
                if h < 4 and h + 4 < HPC:
                    wq_sb[h + 4] = wq_pool.tile([128, NKT, D], F8E4,
                                                tag="wqh", name=f"wqh{h+4}")
                    nc.gpsimd.dma_start(
                        wq_sb[h + 4][:],
                        wq8[:, (h + 4) * NKT:(h + 5) * NKT, :])
                if h == 2:  # wo arrives while attention still running
                    wo_sb = wo_pool.tile([128, HPC * HID], BF16)
                    HW2 = HPC * HID // 2
                    nc.sync.dma_start(wo_sb[:, 0:HW2], wo[:, 0:HW2])
                    nc.gpsimd.dma_start(wo_sb[:, HW2:], wo[:, HW2:])

        # ---- O projection: own 8-bank PSUM pool, deep pipeline -----------
        with ExitStack() as fin:
            psO = fin.enter_context(tc.tile_pool(name="psO", bufs=8,
                                                 space="PSUM"))
            pre = set(PRE_TILES)
            for st in range(NST):
                for ec in range(HID // 512):
                    po = psO.tile([128, 512], F32, tag="po",
                                  name=f"po{st}_{ec}")
                    h0 = 6 if (st, ec) in pre else 0
                    for h in range(h0, HPC):
                        nc.tensor.matmul(
                            po[:], ctxT[h][:, st * 128:(st + 1) * 128],
                            wo_sb[:, h * HID + ec * 512:h * HID + (ec + 1) * 512],
                            start=(h == h0), stop=(h == HPC - 1))
                    ot = out_pool.tile([128, 512], F32, tag="ot")
                    if (st * 4 + ec) % 2:
                        nc.scalar.copy(ot[:], po[:])
                        nc.gpsimd.dma_start(
                            out[st * 128:(st + 1) * 128,
                                ec * 512:(ec + 1) * 512], ot[:])
                    else:
                        nc.vector.tensor_copy(ot[:], po[:])
                        nc.sync.dma_start(
                            out[st * 128:(st + 1) * 128,
                                ec * 512:(ec + 1) * 512], ot[:])
    nc.finalize()
    return nc


def host_prep(hidden_states, Wq, Wk, Wv, Wo):
    """Pre-transpose/cast/relayout all inputs on the host (bf16 + fp8)."""
    bf = ml_dtypes.bfloat16
    f8 = ml_dtypes.float8_e4m3fn
    xTs, xT8s = [], []
    for b in range(B):
        t = hidden_states[b].T.reshape(NKT, 128, S).transpose(1, 0, 2)
        xTs.append(np.ascontiguousarray(t.astype(bf)).reshape(128, NKT * S))
        xT8s.append(np.ascontiguousarray(
            np.clip(t, -240, 240).astype(f8)).reshape(128, NKT * S))
    halves = []
    for hf in range(2):
        wqh = Wq[:, 1024 * hf:1024 * (hf + 1)].reshape(NKT, 128, HPC, D)
        wqh = np.ascontiguousarray(
            np.clip(wqh.transpose(1, 2, 0, 3) * QSCALE, -240, 240)
            .astype(f8)).reshape(128, HPC * NKT * D)
        wkh = Wk[:, KVD * hf:KVD * (hf + 1)].reshape(NKT, 128, KVD)
        wkh = np.ascontiguousarray(
            wkh.transpose(1, 0, 2).astype(bf)).reshape(128, NKT * KVD)
        wvh = Wv[:, KVD * hf:KVD * (hf + 1)].reshape(NKT, 128, KVD)
        wvh = np.ascontiguousarray(
            wvh.transpose(1, 0, 2).astype(bf)).reshape(128, NKT * KVD)
        woh = Wo[1024 * hf:1024 * (hf + 1), :].reshape(HPC, 128, HID)
        woh = np.ascontiguousarray(
            woh.transpose(1, 0, 2).astype(bf)).reshape(128, HPC * HID)
        halves.append((wqh, wkh, wvh, woh))

    inv_freq = 1.0 / (10000.0 ** (np.arange(0, D, 2, dtype=np.float64) / D))
    t = np.arange(S, dtype=np.float64)
    freqs = np.outer(t, inv_freq)
    emb = np.concatenate([freqs, freqs], -1)
    cosT = np.cos(emb).T
    sinTs_f = np.sin(emb).T.copy()
    sinTs_f[:64] *= -1.0
    p64 = np.zeros((D, D), dtype=np.float64)
    for d in range(D):
        p64[d, (d + 64) % D] = 1.0
    dmask = np.triu(np.ones((128, 128), dtype=np.float64))
    tbl = np.ascontiguousarray(
        np.concatenate([cosT, sinTs_f, p64, dmask], axis=1)).astype(bf)
    return xTs, xT8s, halves, tbl


_CACHE = {}


def kernel(hidden_states, Wq, Wk, Wv, Wo, _trace=False, _tmpdir=None):
    hidden_states = np.ascontiguousarray(hidden_states, dtype=np.float32)
    Wq = np.ascontiguousarray(Wq, dtype=np.float32)
    Wk = np.ascontiguousarray(Wk, dtype=np.float32)
    Wv = np.ascontiguousarray(Wv, dtype=np.float32)
    Wo = np.ascontiguousarray(Wo, dtype=np.float32)

    if "nc" not in _CACHE:
        _CACHE["nc"] = build_kernel()
    nc = _CACHE["nc"]
    xTs, xT8s, halves, tbl = host_prep(hidden_states, Wq, Wk, Wv, Wo)

    in_maps = []
    for cid in range(8):
        b, hf = cid // 2, cid % 2
        wqh, wkh, wvh, woh = halves[hf]
        in_maps.append({
            "xT": xTs[b], "xT8": xT8s[b], "wq8": wqh, "wk": wkh, "wv": wvh,
            "wo": woh, "tbl": tbl,
        })
    res = run_bass_kernel_spmd(nc, in_maps, list(range(8)),
                               trace=_trace, tmpdir=_tmpdir)
    out = np.zeros((B, S, HID), dtype=np.float32)
    for cid in range(8):
        out[cid // 2] += res.results[cid]["out"]
        out[cid // 2][0:640, 0:1024] += res.results[cid]["out2"]
    if _trace:
        return out, res
    return out
